# revision 25
# baseline (speedup 1.0000x reference)
"""Trainium2 Bass kernel for the BaselineNCDE problem.

Reference math (per batch row b):
    cp[t] = [time[t], features[t]]                       (C=7)
    h0    = Wp @ features[0] + bp                        (H=64)
    U[t]  = (cp[min(t+1,T-1)] - cp[t]) * (t < length)    [host-folded: equals
            dxdt*dt*active of the reference exactly]
    scan: x1 = gelu(W1 h + b1); x2 = gelu(W2 x1 + b2)
          M  = tanh(W3 x2 + b3) as (H, C);  h += M @ U[t]
          pred[t] = Wr2 relu(Wr1 h + br1) + br2

Device trick: W1@h and Wr1@h are LINEAR in h, so instead of materializing h
we keep y1 = W1@h and r1 = Wr1@h as persistent PSUM accumulators:
    y1 += G1_j @ P_j   with G1_j = (A_j @ W1.T),  P = tanh(Y3) * U_bcast
    r1 += R1_j @ P_j   with R1_j = (A_j @ Wr1.T)
where A_j is the 0/1 c-contraction pattern for rows [112j, 112j+112).
This removes the h update + mm1 from the critical chain.

Layout per core: BS=512 batch on the free axis, 2 streams of NB=256; feature
dims on partitions.  PE: all matmuls (fp32r, 1 cyc/col at N>=256).  ACT:
gelu/tanh.  DVE: the M*U multiply (U replicated over 112 partitions via a
stride-0 DMA), relu, pred evacuation.
"""

import numpy as np

B, T, F = 4096, 256, 6
H, W = 64, 128
C = F + 1            # 7
HC = H * C           # 448
NCORES = 8
BS = B // NCORES     # 512 batch rows per core
NB = 256             # stream width (batch cols per stream)
NS = 2               # pipelined streams
CH = 112             # mm3 chunk rows (448 = 4*112; 112 % 7 == 0)
NCHUNK = 4
NBANK = 2            # psum banks for Y3 (2 chunks each)
R1 = 32              # readout hidden

_BUILD_CACHE = {}


def _build(t_steps=T, mult_bf16=False):
    """Build the Bacc module (same program for every core)."""
    key = (t_steps, mult_bf16)
    if key in _BUILD_CACHE:
        return _BUILD_CACHE[key]

    from contextlib import ExitStack

    import concourse.bass as bass
    import concourse.mybir as mybir
    import concourse.tile as tile
    from concourse import bacc

    dt = mybir.dt
    AF = mybir.ActivationFunctionType
    ALU = mybir.AluOpType
    f32 = dt.float32
    f32r = dt.float32r   # fp32 bits; PE fast mode (1 cyc/col at N>=256)
    mdt = dt.bfloat16 if mult_bf16 else f32
    pdt = dt.bfloat16 if mult_bf16 else f32r

    assert t_steps % 4 == 0
    nc = bacc.Bacc("TRN2", target_bir_lowering=False, debug=False)

    # ---- DRAM I/O ----
    u_d = nc.dram_tensor("u", [t_steps, C, BS], mdt, kind="ExternalInput")
    f0_d = nc.dram_tensor("f0t", [F, BS], f32r, kind="ExternalInput")
    w2t_d = nc.dram_tensor("w2t", [W, W], f32r, kind="ExternalInput")
    w3t_d = nc.dram_tensor("w3t", [W, HC], f32r, kind="ExternalInput")
    b1_d = nc.dram_tensor("b1c", [W, 1], f32, kind="ExternalInput")
    b2_d = nc.dram_tensor("b2c", [W, 1], f32, kind="ExternalInput")
    b3p_d = nc.dram_tensor("b3p", [2 * NBANK, CH], f32r, kind="ExternalInput")
    o01_d = nc.dram_tensor("o01", [2, 2 * NB], f32r, kind="ExternalInput")
    g1_d = nc.dram_tensor("g1", [CH, NCHUNK * W], f32r, kind="ExternalInput")
    r1p_d = nc.dram_tensor("r1p", [CH, NCHUNK * R1], f32r, kind="ExternalInput")
    wq1_d = nc.dram_tensor("wq1", [F, W], f32r, kind="ExternalInput")
    bq1_d = nc.dram_tensor("bq1", [1, W], f32r, kind="ExternalInput")
    wqr_d = nc.dram_tensor("wqr", [F, R1], f32r, kind="ExternalInput")
    bqr_d = nc.dram_tensor("bqr", [1, R1], f32r, kind="ExternalInput")
    br1_d = nc.dram_tensor("br1c", [R1, 1], f32, kind="ExternalInput")
    wr2t_d = nc.dram_tensor("wr2t", [R1, 1], f32r, kind="ExternalInput")
    br2_d = nc.dram_tensor("br2c", [4, 1], f32, kind="ExternalInput")
    pred_d = nc.dram_tensor("pred", [t_steps, BS], f32, kind="ExternalOutput")

    with tile.TileContext(nc) as tc, ExitStack() as ctx:
        const = ctx.enter_context(tc.tile_pool(name="const", bufs=1))

        def load_const(dram, shape, dtype, tag):
            t_ = const.tile(shape, dtype, tag=tag, name=tag)
            nc.sync.dma_start(t_[:], dram.ap())
            return t_

        w2t_s = load_const(w2t_d, [W, W], f32r, "w2t")
        w3t_s = load_const(w3t_d, [W, HC], f32r, "w3t")
        b1_s = load_const(b1_d, [W, 1], f32, "b1")
        b2_s = load_const(b2_d, [W, 1], f32, "b2")
        # bias pair rows per bank at 32-aligned partitions: bank b at 32*b
        b3p_s = const.tile([34, CH], f32r, tag="b3p", name="b3p")
        nc.sync.dma_start(b3p_s[0:2, :], b3p_d.ap()[0:2, :])
        nc.sync.dma_start(b3p_s[32:34, :], b3p_d.ap()[2:4, :])
        o01_s = const.tile([34, 2 * NB], f32r, tag="o01", name="o01")
        nc.sync.dma_start(o01_s[0:2, :], o01_d.ap())
        nc.sync.dma_start(o01_s[32:34, :], o01_d.ap())
        g1_s = load_const(g1_d, [CH, NCHUNK * W], f32r, "g1")
        r1p_s = load_const(r1p_d, [CH, NCHUNK * R1], f32r, "r1p")
        wq1_s = load_const(wq1_d, [F, W], f32r, "wq1")
        bq1_s = load_const(bq1_d, [1, W], f32r, "bq1")
        wqr_s = load_const(wqr_d, [F, R1], f32r, "wqr")
        bqr_s = load_const(bqr_d, [1, R1], f32r, "bqr")
        br1_s = load_const(br1_d, [R1, 1], f32, "br1")
        wr2t_s = load_const(wr2t_d, [R1, 1], f32r, "wr2t")
        br2_s = load_const(br2_d, [4, 1], f32, "br2")
        f0_s = load_const(f0_d, [F, BS], f32r, "f0")
        ones_s = const.tile([1, NB], f32r, tag="ones", name="ones")
        nc.vector.memset(ones_s[:], 1.0)

        u_pool = ctx.enter_context(tc.tile_pool(name="upool", bufs=3))
        x_pool = ctx.enter_context(tc.tile_pool(name="xpool", bufs=2))
        m_pool = ctx.enter_context(tc.tile_pool(name="mpool", bufs=2))
        r_pool = ctx.enter_context(tc.tile_pool(name="rpool", bufs=2))
        pred_pool = ctx.enter_context(tc.tile_pool(name="predpool", bufs=1))

        # PSUM: per stream: acc (y1+r1, persistent) 1 bank, y3 2 banks,
        # transient (y2 | r2 rows) 1 bank  -> 4 banks * 2 streams = 8.
        ps_acc, ps_y3, ps_tr = [], [], []
        for s in range(NS):
            ps_acc.append(
                ctx.enter_context(
                    tc.tile_pool(name=f"psacc{s}", bufs=1, space="PSUM")
                )
            )
            ps_y3.append(
                ctx.enter_context(tc.tile_pool(name=f"psy3{s}", bufs=1, space="PSUM"))
            )
            ps_tr.append(
                ctx.enter_context(tc.tile_pool(name=f"pstr{s}", bufs=1, space="PSUM"))
            )
        r2acc = [
            ps_tr[s].tile([4 * R1, NB], f32, tag=f"r2acc{s}", name=f"r2acc{s}")
            for s in range(NS)
        ]
        acc = [
            ps_acc[s].tile([W, 2 * NB], f32, tag=f"acc{s}", name=f"acc{s}")
            for s in range(NS)
        ]
        # acc[s][0:128, 0:NB] = y1 = W1 @ h;  acc[s][0:32, NB:2NB] = r1 = Wr1 @ h

        # ---- seed accumulators: y1 = W1@Wp@f0 + W1@bp, r1 = Wr1@Wp@f0 + Wr1@bp
        for s in range(NS):
            fsl = f0_s[:, s * NB : (s + 1) * NB]
            y1r = acc[s][:, 0:NB]
            r1r = acc[s][0:R1, NB : NB + NB]
            nc.tensor.matmul(y1r, wq1_s[:], fsl, start=True, stop=False)
            nc.tensor.matmul(y1r, bq1_s[:], ones_s[:], start=False, stop=True)
            nc.tensor.matmul(
                r1r, wqr_s[:], fsl, start=False, stop=False, skip_group_check=True
            )
            nc.tensor.matmul(
                r1r, bqr_s[:], ones_s[:], start=False, stop=True,
                skip_group_check=True,
            )

        # pred staging: partition = t % 4, columns = (t//4)*BS + b
        n_g = t_steps // 4
        pred_sb = pred_pool.tile([4, n_g * BS], f32, tag="pred", name="pred_sb")

        skew = {}

        def emit_step(s, t):
            u_t = u_pool.tile([CH, NB], mdt, tag=f"u{s}", name=f"u{s}_{t}")
            nc.sync.dma_start(
                u_t[:],
                bass.AP(u_d, (t * C) * BS + s * NB, [[0, CH // C], [BS, C], [1, NB]]),
            )

            # x1 = gelu(y1acc + b1)
            x1 = x_pool.tile([W, NB], f32r, tag=f"x1{s}", name=f"x1_{s}_{t}")
            g1i = nc.scalar.activation(
                x1[:], acc[s][:, 0:NB], AF.Gelu_apprx_tanh, bias=b1_s[:]
            )
            other = 1 - s
            if other in skew:
                tile.add_dep_helper(
                    g1i.ins, skew[other].ins, sync=True,
                    reason="stream anti-phase lock",
                )

            # y2 = W2 @ x1 ; x2 = gelu(y2 + b2)   (y2 borrows the y3-bank0 slot)
            y2 = ps_y3[s].tile([W, NB], f32, tag=f"y3{s}b0", name=f"y2_{s}_{t}")
            nc.tensor.matmul(y2[:], w2t_s[:], x1[:], start=True, stop=True)
            x2 = x_pool.tile([W, NB], f32r, tag=f"x2{s}", name=f"x2_{s}_{t}")
            g2i = nc.scalar.activation(
                x2[:], y2[:], AF.Gelu_apprx_tanh, bias=b2_s[:]
            )
            skew[s] = g2i

            # per-bank wavefront: Y3 bank -> tanh -> P = M*U -> y1/r1 accum
            for bank in range(NBANK):
                j0 = 2 * bank
                y3 = ps_y3[s].tile(
                    [CH, 2 * NB], f32, tag=f"y3{s}b{bank}", name=f"y3_{s}_{bank}_{t}"
                )
                nc.tensor.matmul(
                    y3[:],
                    b3p_s[32 * bank : 32 * bank + 2, :],
                    o01_s[32 * bank : 32 * bank + 2, :],
                    start=True,
                    stop=False,
                )
                for idx, j in enumerate((j0, j0 + 1)):
                    nc.tensor.matmul(
                        y3[:, idx * NB : (idx + 1) * NB],
                        w3t_s[:, j * CH : (j + 1) * CH],
                        x2[:],
                        start=False,
                        stop=(idx == 1),
                    )
                m_t = m_pool.tile(
                    [CH, 2 * NB], mdt, tag=f"m{s}b{bank}", name=f"m_{s}_{bank}_{t}"
                )
                nc.scalar.activation(m_t[:], y3[:], AF.Tanh)
                p_t = m_pool.tile(
                    [CH, 2 * NB], pdt, tag=f"p{s}b{bank}", name=f"p_{s}_{bank}_{t}"
                )
                m3 = m_t[:].rearrange("p (j n) -> p j n", j=2)
                p3 = p_t[:].rearrange("p (j n) -> p j n", j=2)
                u3 = bass.AP(
                    u_t.tensor, u_t.offset, [list(u_t.ap[0]), [0, 2], [1, NB]]
                )
                nc.vector.tensor_tensor(p3, m3, u3, op=ALU.mult)
                for idx, j in enumerate((j0, j0 + 1)):
                    psl = p_t[:, idx * NB : (idx + 1) * NB]
                    last_mm = bank == NBANK - 1 and idx == 1
                    nc.tensor.matmul(
                        acc[s][:, 0:NB],
                        g1_s[:, j * W : (j + 1) * W],
                        psl,
                        start=False,
                        stop=last_mm,
                        skip_group_check=True,
                    )
                    nc.tensor.matmul(
                        acc[s][0:R1, NB : NB + NB],
                        r1p_s[:, j * R1 : (j + 1) * R1],
                        psl,
                        start=False,
                        stop=last_mm,
                        skip_group_check=True,
                    )

            # readout: rl = relu(r1acc + br1); r2 = Wr2 @ rl (+ br2 at evac)
            rl = r_pool.tile([R1, NB], f32r, tag=f"rl{s}", name=f"rl_{s}_{t}")
            nc.vector.tensor_scalar(
                rl[:], acc[s][0:R1, NB : NB + NB], br1_s[:], 0.0,
                op0=ALU.add, op1=ALU.max,
            )
            q = t % 4
            nc.tensor.matmul(
                r2acc[s][32 * q : 32 * q + 1, :],
                wr2t_s[:],
                rl[:],
                start=True,
                stop=True,
                tile_position=(0, 32 * q),
            )
            if q == 3:
                g = t // 4
                src = bass.AP(
                    r2acc[s].tensor,
                    r2acc[s].offset,
                    [[32 * r2acc[s].ap[0][0], 4], [1, NB]],
                )
                dst = pred_sb[0:4, g * BS + s * NB : g * BS + (s + 1) * NB]
                nc.vector.tensor_scalar(dst, src, br2_s[:], None, op0=ALU.add)

        for t in range(t_steps):
            for s in range(NS):
                emit_step(s, t)

        # final: pred_sb (4, G*BS) -> pred_d (T, BS), t = 4g + p
        nc.sync.dma_start(
            bass.AP(pred_d, 0, [[BS, 4], [4 * BS, n_g], [1, BS]]),
            pred_sb[:].rearrange("p (g b) -> p g b", b=BS),
        )

    nc.compile()
    _BUILD_CACHE[key] = (nc, None)
    return nc, None


def _host_prep(time, features, mask, length, Wp, bp, W1, b1, W2, b2, W3, b3,
               Wr1, br1, Wr2, br2, t_steps=T, mult_bf16=False):
    """Shard + marshal inputs into per-core in_maps."""
    time = np.asarray(time, np.float32)
    features = np.asarray(features, np.float32)
    W1, W2, W3 = (np.asarray(x, np.float32) for x in (W1, W2, W3))
    Wp, Wr1, Wr2 = (np.asarray(x, np.float32) for x in (Wp, Wr1, Wr2))
    b1, b2, b3 = (np.asarray(x, np.float32) for x in (b1, b2, b3))
    bp, br1, br2 = (np.asarray(x, np.float32) for x in (bp, br1, br2))
    mdt = np.float32
    if mult_bf16:
        import ml_dtypes

        mdt = ml_dtypes.bfloat16

    cp = np.concatenate([time[..., None], features], axis=-1)  # (B, Tfull, C)
    cp_next = np.concatenate([cp[:, 1:], cp[:, -1:]], axis=1)
    active = np.arange(cp.shape[1])[None, :] < np.asarray(length)[:, None]
    u_full = ((cp_next - cp) * active[..., None])[:, :t_steps].astype(np.float32)

    # c-contraction patterns A_j (112, 64): A_j[p, h] = 1 iff (112j+p)//7 == h
    a_list = []
    for j in range(NCHUNK):
        a = np.zeros((CH, H), np.float32)
        for p in range(CH):
            a[p, (CH * j + p) // C] = 1.0
        a_list.append(a)

    g1 = np.concatenate([a @ W1.T for a in a_list], axis=1)     # (112, 4*128)
    r1p = np.concatenate([a @ Wr1.T for a in a_list], axis=1)   # (112, 4*32)

    b3p = np.zeros((2 * NBANK, CH), np.float32)                 # rank-2 bias rows
    for bank in range(NBANK):
        b3p[2 * bank] = b3[(2 * bank) * CH : (2 * bank + 1) * CH]
        b3p[2 * bank + 1] = b3[(2 * bank + 1) * CH : (2 * bank + 2) * CH]
    o01 = np.zeros((2, 2 * NB), np.float32)
    o01[0, :NB] = 1.0
    o01[1, NB:] = 1.0

    shared = {
        "w2t": np.ascontiguousarray(W2.T),
        "w3t": np.ascontiguousarray(W3.T),
        "b1c": np.ascontiguousarray(b1.reshape(W, 1)),
        "b2c": np.ascontiguousarray(b2.reshape(W, 1)),
        "b3p": b3p,
        "o01": o01,
        "g1": np.ascontiguousarray(g1),
        "r1p": np.ascontiguousarray(r1p),
        "wq1": np.ascontiguousarray((W1 @ Wp).T),               # (6, 128)
        "bq1": np.ascontiguousarray((W1 @ bp).reshape(1, W)),
        "wqr": np.ascontiguousarray((Wr1 @ Wp).T),              # (6, 32)
        "bqr": np.ascontiguousarray((Wr1 @ bp).reshape(1, R1)),
        "br1c": np.ascontiguousarray(br1.reshape(R1, 1)),
        "wr2t": np.ascontiguousarray(Wr2.T),
        "br2c": np.full((4, 1), np.float32(br2.reshape(-1)[0]), np.float32),
    }

    in_maps = []
    for i in range(NCORES):
        bsl = slice(i * BS, (i + 1) * BS)
        m = dict(shared)
        m["u"] = np.ascontiguousarray(u_full[bsl].transpose(1, 2, 0)).astype(mdt)
        m["f0t"] = np.ascontiguousarray(features[bsl, 0, :].T)
        in_maps.append(m)
    return in_maps


def kernel(**inputs):
    from concourse.bass_utils import run_bass_kernel_spmd

    nc, _ = _build(t_steps=T, mult_bf16=False)
    in_maps = _host_prep(**inputs, t_steps=T, mult_bf16=False)
    res = run_bass_kernel_spmd(nc, in_maps, list(range(NCORES)))
    preds = [res.results[i]["pred"] for i in range(NCORES)]  # (T, BS) each
    out = np.concatenate([p.T for p in preds], axis=0)  # (B, T)
    return np.ascontiguousarray(out.astype(np.float32))


# revision 26
# speedup vs baseline: 7.6263x; 7.6263x over previous
"""Trainium2 Bass kernel for the BaselineNCDE problem.

Reference math (per batch row b):
    cp[t] = [time[t], features[t]]                       (C=7)
    h0    = Wp @ features[0] + bp                        (H=64)
    U[t]  = (cp[min(t+1,T-1)] - cp[t]) * (t < length)    [host-folded: equals
            dxdt*dt*active of the reference exactly]
    scan: x1 = gelu(W1 h + b1); x2 = gelu(W2 x1 + b2)
          M  = tanh(W3 x2 + b3) as (H, C);  h += M @ U[t]
          pred[t] = Wr2 relu(Wr1 h + br1) + br2

Device trick: W1@h and Wr1@h are LINEAR in h, so instead of materializing h
we keep y1 = W1@h and r1 = Wr1@h as persistent PSUM accumulators:
    y1 += G1_j @ P_j   with G1_j = (A_j @ W1.T),  P = tanh(Y3) * U_bcast
    r1 += R1_j @ P_j   with R1_j = (A_j @ Wr1.T)
where A_j is the 0/1 c-contraction pattern for rows [112j, 112j+112).
This removes the h update + mm1 from the critical chain.

Layout per core: BS=512 batch on the free axis, 2 streams of NB=256; feature
dims on partitions.  PE: all matmuls (fp32r, 1 cyc/col at N>=256).  ACT:
gelu/tanh.  DVE: the M*U multiply (U replicated over 112 partitions via a
stride-0 DMA), relu, pred evacuation.
"""

import numpy as np

B, T, F = 4096, 256, 6
H, W = 64, 128
C = F + 1            # 7
HC = H * C           # 448
NCORES = 8
BS = B // NCORES     # 512 batch rows per core
NB = 256             # stream width (batch cols per stream)
NS = 2               # pipelined streams
CH = 112             # mm3 chunk rows (448 = 4*112; 112 % 7 == 0)
NCHUNK = 4
NBANK = 2            # psum banks for Y3 (2 chunks each)
R1 = 32              # readout hidden

_BUILD_CACHE = {}


def _build(t_steps=T, mult_bf16=False):
    """Build the Bacc module (same program for every core)."""
    key = (t_steps, mult_bf16)
    if key in _BUILD_CACHE:
        return _BUILD_CACHE[key]

    from contextlib import ExitStack

    import concourse.bass as bass
    import concourse.mybir as mybir
    import concourse.tile as tile
    from concourse import bacc

    dt = mybir.dt
    AF = mybir.ActivationFunctionType
    ALU = mybir.AluOpType
    f32 = dt.float32
    f32r = dt.float32r   # fp32 bits; PE fast mode (1 cyc/col at N>=256)
    mdt = dt.bfloat16 if mult_bf16 else f32
    pdt = dt.bfloat16 if mult_bf16 else f32r

    assert t_steps % 4 == 0
    nc = bacc.Bacc("TRN2", target_bir_lowering=False, debug=False)

    # ---- DRAM I/O ----
    u_d = nc.dram_tensor("u", [t_steps, C, BS], mdt, kind="ExternalInput")
    f0_d = nc.dram_tensor("f0t", [F, BS], f32r, kind="ExternalInput")
    w2t_d = nc.dram_tensor("w2t", [W, W], f32r, kind="ExternalInput")
    w3t_d = nc.dram_tensor("w3t", [W, HC], f32r, kind="ExternalInput")
    b1_d = nc.dram_tensor("b1c", [W, 1], f32, kind="ExternalInput")
    b2_d = nc.dram_tensor("b2c", [W, 1], f32, kind="ExternalInput")
    b3p_d = nc.dram_tensor("b3p", [2 * NBANK, CH], f32r, kind="ExternalInput")
    o01_d = nc.dram_tensor("o01", [2, 2 * NB], f32r, kind="ExternalInput")
    g1_d = nc.dram_tensor("g1", [CH, NCHUNK * W], f32r, kind="ExternalInput")
    r1p_d = nc.dram_tensor("r1p", [CH, NCHUNK * R1], f32r, kind="ExternalInput")
    wq1_d = nc.dram_tensor("wq1", [F, W], f32r, kind="ExternalInput")
    bq1_d = nc.dram_tensor("bq1", [1, W], f32r, kind="ExternalInput")
    wqr_d = nc.dram_tensor("wqr", [F, R1], f32r, kind="ExternalInput")
    bqr_d = nc.dram_tensor("bqr", [1, R1], f32r, kind="ExternalInput")
    br1_d = nc.dram_tensor("br1c", [R1, 1], f32, kind="ExternalInput")
    wr2t_d = nc.dram_tensor("wr2t", [R1, 1], f32r, kind="ExternalInput")
    br2_d = nc.dram_tensor("br2c", [4, 1], f32, kind="ExternalInput")
    pred_d = nc.dram_tensor("pred", [t_steps, BS], f32, kind="ExternalOutput")

    with tile.TileContext(nc) as tc, ExitStack() as ctx:
        const = ctx.enter_context(tc.tile_pool(name="const", bufs=1))

        def load_const(dram, shape, dtype, tag):
            t_ = const.tile(shape, dtype, tag=tag, name=tag)
            nc.sync.dma_start(t_[:], dram.ap())
            return t_

        w2t_s = load_const(w2t_d, [W, W], f32r, "w2t")
        w3t_s = load_const(w3t_d, [W, HC], f32r, "w3t")
        b1_s = load_const(b1_d, [W, 1], f32, "b1")
        b2_s = load_const(b2_d, [W, 1], f32, "b2")
        # bias pair rows per bank at 32-aligned partitions: bank b at 32*b
        b3p_s = const.tile([34, CH], f32r, tag="b3p", name="b3p")
        nc.sync.dma_start(b3p_s[0:2, :], b3p_d.ap()[0:2, :])
        nc.sync.dma_start(b3p_s[32:34, :], b3p_d.ap()[2:4, :])
        o01_s = const.tile([34, 2 * NB], f32r, tag="o01", name="o01")
        nc.sync.dma_start(o01_s[0:2, :], o01_d.ap())
        nc.sync.dma_start(o01_s[32:34, :], o01_d.ap())
        g1_s = load_const(g1_d, [CH, NCHUNK * W], f32r, "g1")
        r1p_s = load_const(r1p_d, [CH, NCHUNK * R1], f32r, "r1p")
        wq1_s = load_const(wq1_d, [F, W], f32r, "wq1")
        bq1_s = load_const(bq1_d, [1, W], f32r, "bq1")
        wqr_s = load_const(wqr_d, [F, R1], f32r, "wqr")
        bqr_s = load_const(bqr_d, [1, R1], f32r, "bqr")
        br1_s = load_const(br1_d, [R1, 1], f32, "br1")
        wr2t_s = load_const(wr2t_d, [R1, 1], f32r, "wr2t")
        br2_s = load_const(br2_d, [4, 1], f32, "br2")
        f0_s = load_const(f0_d, [F, BS], f32r, "f0")
        ones_s = const.tile([1, NB], f32r, tag="ones", name="ones")
        nc.vector.memset(ones_s[:], 1.0)

        u_pool = ctx.enter_context(tc.tile_pool(name="upool", bufs=3))
        x_pool = ctx.enter_context(tc.tile_pool(name="xpool", bufs=2))
        m_pool = ctx.enter_context(tc.tile_pool(name="mpool", bufs=2))
        r_pool = ctx.enter_context(tc.tile_pool(name="rpool", bufs=2))
        pred_pool = ctx.enter_context(tc.tile_pool(name="predpool", bufs=1))

        # PSUM: per stream: acc (y1+r1, persistent) 1 bank, y3 2 banks,
        # transient (y2 | r2 rows) 1 bank  -> 4 banks * 2 streams = 8.
        ps_acc, ps_y3, ps_tr = [], [], []
        for s in range(NS):
            ps_acc.append(
                ctx.enter_context(
                    tc.tile_pool(name=f"psacc{s}", bufs=1, space="PSUM")
                )
            )
            ps_y3.append(
                ctx.enter_context(tc.tile_pool(name=f"psy3{s}", bufs=1, space="PSUM"))
            )
            ps_tr.append(
                ctx.enter_context(tc.tile_pool(name=f"pstr{s}", bufs=1, space="PSUM"))
            )
        r2acc = [
            ps_tr[s].tile([4 * R1, NB], f32, tag=f"r2acc{s}", name=f"r2acc{s}")
            for s in range(NS)
        ]
        acc = [
            ps_acc[s].tile([W, 2 * NB], f32, tag=f"acc{s}", name=f"acc{s}")
            for s in range(NS)
        ]
        # acc[s][0:128, 0:NB] = y1 = W1 @ h;  acc[s][0:32, NB:2NB] = r1 = Wr1 @ h

        # ---- seed accumulators: y1 = W1@Wp@f0 + W1@bp, r1 = Wr1@Wp@f0 + Wr1@bp
        for s in range(NS):
            fsl = f0_s[:, s * NB : (s + 1) * NB]
            y1r = acc[s][:, 0:NB]
            r1r = acc[s][0:R1, NB : NB + NB]
            nc.tensor.matmul(y1r, wq1_s[:], fsl, start=True, stop=False)
            nc.tensor.matmul(y1r, bq1_s[:], ones_s[:], start=False, stop=True)
            nc.tensor.matmul(
                r1r, wqr_s[:], fsl, start=False, stop=False, skip_group_check=True
            )
            nc.tensor.matmul(
                r1r, bqr_s[:], ones_s[:], start=False, stop=True,
                skip_group_check=True,
            )

        # pred staging: partition = 32*(t%4), columns = (t//4)*BS + b
        n_g = t_steps // 4
        pred_sb = pred_pool.tile([97, n_g * BS], f32, tag="pred", name="pred_sb")

        skew = {}

        def emit_step(s, t):
            u_t = u_pool.tile([CH, NB], mdt, tag=f"u{s}", name=f"u{s}_{t}")
            nc.sync.dma_start(
                u_t[:],
                bass.AP(u_d, (t * C) * BS + s * NB, [[0, CH // C], [BS, C], [1, NB]]),
            )

            # x1 = gelu(y1acc + b1)
            x1 = x_pool.tile([W, NB], f32r, tag=f"x1{s}", name=f"x1_{s}_{t}")
            g1i = nc.scalar.activation(
                x1[:], acc[s][:, 0:NB], AF.Gelu_apprx_tanh, bias=b1_s[:]
            )
            other = 1 - s
            if other in skew:
                tile.add_dep_helper(
                    g1i.ins, skew[other].ins, sync=True,
                    reason="stream anti-phase lock",
                )

            # y2 = W2 @ x1 ; x2 = gelu(y2 + b2)   (y2 borrows the y3-bank0 slot)
            y2 = ps_y3[s].tile([W, NB], f32, tag=f"y3{s}b0", name=f"y2_{s}_{t}")
            nc.tensor.matmul(y2[:], w2t_s[:], x1[:], start=True, stop=True)
            x2 = x_pool.tile([W, NB], f32r, tag=f"x2{s}", name=f"x2_{s}_{t}")
            g2i = nc.scalar.activation(
                x2[:], y2[:], AF.Gelu_apprx_tanh, bias=b2_s[:]
            )
            skew[s] = g2i

            # per-bank wavefront: Y3 bank -> tanh -> P = M*U -> y1/r1 accum
            for bank in range(NBANK):
                j0 = 2 * bank
                y3 = ps_y3[s].tile(
                    [CH, 2 * NB], f32, tag=f"y3{s}b{bank}", name=f"y3_{s}_{bank}_{t}"
                )
                nc.tensor.matmul(
                    y3[:],
                    b3p_s[32 * bank : 32 * bank + 2, :],
                    o01_s[32 * bank : 32 * bank + 2, :],
                    start=True,
                    stop=False,
                )
                for idx, j in enumerate((j0, j0 + 1)):
                    nc.tensor.matmul(
                        y3[:, idx * NB : (idx + 1) * NB],
                        w3t_s[:, j * CH : (j + 1) * CH],
                        x2[:],
                        start=False,
                        stop=(idx == 1),
                    )
                m_t = m_pool.tile(
                    [CH, 2 * NB], mdt, tag=f"m{s}b{bank}", name=f"m_{s}_{bank}_{t}"
                )
                nc.scalar.activation(m_t[:], y3[:], AF.Tanh)
                p_t = m_pool.tile(
                    [CH, 2 * NB], pdt, tag=f"p{s}b{bank}", name=f"p_{s}_{bank}_{t}"
                )
                m3 = m_t[:].rearrange("p (j n) -> p j n", j=2)
                p3 = p_t[:].rearrange("p (j n) -> p j n", j=2)
                u3 = bass.AP(
                    u_t.tensor, u_t.offset, [list(u_t.ap[0]), [0, 2], [1, NB]]
                )
                nc.vector.tensor_tensor(p3, m3, u3, op=ALU.mult)
                for idx, j in enumerate((j0, j0 + 1)):
                    psl = p_t[:, idx * NB : (idx + 1) * NB]
                    last_mm = bank == NBANK - 1 and idx == 1
                    nc.tensor.matmul(
                        acc[s][:, 0:NB],
                        g1_s[:, j * W : (j + 1) * W],
                        psl,
                        start=False,
                        stop=last_mm,
                        skip_group_check=True,
                    )
                    nc.tensor.matmul(
                        acc[s][0:R1, NB : NB + NB],
                        r1p_s[:, j * R1 : (j + 1) * R1],
                        psl,
                        start=False,
                        stop=last_mm,
                        skip_group_check=True,
                    )

            # readout: rl = relu(r1acc + br1); r2 = Wr2 @ rl (+ br2 at evac)
            rl = r_pool.tile([R1, NB], f32r, tag=f"rl{s}", name=f"rl_{s}_{t}")
            nc.vector.tensor_scalar(
                rl[:], acc[s][0:R1, NB : NB + NB], br1_s[:], 0.0,
                op0=ALU.add, op1=ALU.max,
            )
            q = t % 4
            g = t // 4
            nc.tensor.matmul(
                r2acc[s][32 * q : 32 * q + 1, :],
                wr2t_s[:],
                rl[:],
                start=True,
                stop=True,
                tile_position=(0, 32 * q),
            )
            dst = pred_sb[32 * q : 32 * q + 1, g * BS + s * NB : g * BS + (s + 1) * NB]
            nc.vector.tensor_scalar(
                dst, r2acc[s][32 * q : 32 * q + 1, :], br2_s[0:1, :], None, op0=ALU.add
            )

        for t in range(t_steps):
            for s in range(NS):
                emit_step(s, t)

        # final: pred_sb row 32q holds steps t = 4g+q -> pred_d (T, BS)
        for q in range(4):
            nc.sync.dma_start(
                bass.AP(pred_d, q * BS, [[4 * BS, n_g], [1, BS]]),
                pred_sb[32 * q : 32 * q + 1, :].rearrange("p (g b) -> p g b", b=BS),
            )

    nc.compile()
    _BUILD_CACHE[key] = (nc, None)
    return nc, None


def _host_prep(time, features, mask, length, Wp, bp, W1, b1, W2, b2, W3, b3,
               Wr1, br1, Wr2, br2, t_steps=T, mult_bf16=False):
    """Shard + marshal inputs into per-core in_maps."""
    time = np.asarray(time, np.float32)
    features = np.asarray(features, np.float32)
    W1, W2, W3 = (np.asarray(x, np.float32) for x in (W1, W2, W3))
    Wp, Wr1, Wr2 = (np.asarray(x, np.float32) for x in (Wp, Wr1, Wr2))
    b1, b2, b3 = (np.asarray(x, np.float32) for x in (b1, b2, b3))
    bp, br1, br2 = (np.asarray(x, np.float32) for x in (bp, br1, br2))
    mdt = np.float32
    if mult_bf16:
        import ml_dtypes

        mdt = ml_dtypes.bfloat16

    cp = np.concatenate([time[..., None], features], axis=-1)  # (B, Tfull, C)
    cp_next = np.concatenate([cp[:, 1:], cp[:, -1:]], axis=1)
    active = np.arange(cp.shape[1])[None, :] < np.asarray(length)[:, None]
    u_full = ((cp_next - cp) * active[..., None])[:, :t_steps].astype(np.float32)

    # c-contraction patterns A_j (112, 64): A_j[p, h] = 1 iff (112j+p)//7 == h
    a_list = []
    for j in range(NCHUNK):
        a = np.zeros((CH, H), np.float32)
        for p in range(CH):
            a[p, (CH * j + p) // C] = 1.0
        a_list.append(a)

    g1 = np.concatenate([a @ W1.T for a in a_list], axis=1)     # (112, 4*128)
    r1p = np.concatenate([a @ Wr1.T for a in a_list], axis=1)   # (112, 4*32)

    b3p = np.zeros((2 * NBANK, CH), np.float32)                 # rank-2 bias rows
    for bank in range(NBANK):
        b3p[2 * bank] = b3[(2 * bank) * CH : (2 * bank + 1) * CH]
        b3p[2 * bank + 1] = b3[(2 * bank + 1) * CH : (2 * bank + 2) * CH]
    o01 = np.zeros((2, 2 * NB), np.float32)
    o01[0, :NB] = 1.0
    o01[1, NB:] = 1.0

    shared = {
        "w2t": np.ascontiguousarray(W2.T),
        "w3t": np.ascontiguousarray(W3.T),
        "b1c": np.ascontiguousarray(b1.reshape(W, 1)),
        "b2c": np.ascontiguousarray(b2.reshape(W, 1)),
        "b3p": b3p,
        "o01": o01,
        "g1": np.ascontiguousarray(g1),
        "r1p": np.ascontiguousarray(r1p),
        "wq1": np.ascontiguousarray((W1 @ Wp).T),               # (6, 128)
        "bq1": np.ascontiguousarray((W1 @ bp).reshape(1, W)),
        "wqr": np.ascontiguousarray((Wr1 @ Wp).T),              # (6, 32)
        "bqr": np.ascontiguousarray((Wr1 @ bp).reshape(1, R1)),
        "br1c": np.ascontiguousarray(br1.reshape(R1, 1)),
        "wr2t": np.ascontiguousarray(Wr2.T),
        "br2c": np.full((4, 1), np.float32(br2.reshape(-1)[0]), np.float32),
    }

    in_maps = []
    for i in range(NCORES):
        bsl = slice(i * BS, (i + 1) * BS)
        m = dict(shared)
        m["u"] = np.ascontiguousarray(u_full[bsl].transpose(1, 2, 0)).astype(mdt)
        m["f0t"] = np.ascontiguousarray(features[bsl, 0, :].T)
        in_maps.append(m)
    return in_maps


def kernel(**inputs):
    from concourse.bass_utils import run_bass_kernel_spmd

    nc, _ = _build(t_steps=T, mult_bf16=False)
    in_maps = _host_prep(**inputs, t_steps=T, mult_bf16=False)
    res = run_bass_kernel_spmd(nc, in_maps, list(range(NCORES)))
    preds = [res.results[i]["pred"] for i in range(NCORES)]  # (T, BS) each
    out = np.concatenate([p.T for p in preds], axis=0)  # (B, T)
    return np.ascontiguousarray(out.astype(np.float32))


# revision 28
# speedup vs baseline: 24.3022x; 3.1866x over previous
"""Trainium2 Bass kernel for the BaselineNCDE problem.

Reference math (per batch row b):
    cp[t] = [time[t], features[t]]                       (C=7)
    h0    = Wp @ features[0] + bp                        (H=64)
    U[t]  = (cp[min(t+1,T-1)] - cp[t]) * (t < length)    [host-folded: equals
            dxdt*dt*active of the reference exactly]
    scan: x1 = gelu(W1 h + b1); x2 = gelu(W2 x1 + b2)
          M  = tanh(W3 x2 + b3) as (H, C);  h += M @ U[t]
          pred[t] = Wr2 relu(Wr1 h + br1) + br2

Device trick: W1@h and Wr1@h are LINEAR in h, so instead of materializing h
we keep y1 = W1@h and r1 = Wr1@h as persistent PSUM accumulators:
    y1 += G1_j @ P_j   with G1_j = (A_j @ W1.T),  P = tanh(Y3) * U_bcast
    r1 += R1_j @ P_j   with R1_j = (A_j @ Wr1.T)
where A_j is the 0/1 c-contraction pattern for rows [112j, 112j+112).
This removes the h update + mm1 from the critical chain.

Layout per core: BS=512 batch on the free axis, 2 streams of NB=256; feature
dims on partitions.  PE: all matmuls (fp32r, 1 cyc/col at N>=256).  ACT:
gelu/tanh.  DVE: the M*U multiply (U replicated over 112 partitions via a
stride-0 DMA), relu, pred evacuation.
"""

import numpy as np

B, T, F = 4096, 256, 6
H, W = 64, 128
C = F + 1            # 7
HC = H * C           # 448
NCORES = 8
BS = B // NCORES     # 512 batch rows per core
NB = 256             # stream width (batch cols per stream)
NS = 2               # pipelined streams
CH = 112             # mm3 chunk rows (448 = 4*112; 112 % 7 == 0)
NCHUNK = 4
NBANK = 2            # psum banks for Y3 (2 chunks each)
R1 = 32              # readout hidden

_BUILD_CACHE = {}


def _build(t_steps=T, mult_bf16=False):
    """Build the Bacc module (same program for every core)."""
    key = (t_steps, mult_bf16)
    if key in _BUILD_CACHE:
        return _BUILD_CACHE[key]

    from contextlib import ExitStack

    import concourse.bass as bass
    import concourse.mybir as mybir
    import concourse.tile as tile
    from concourse import bacc

    dt = mybir.dt
    AF = mybir.ActivationFunctionType
    ALU = mybir.AluOpType
    f32 = dt.float32
    f32r = dt.float32r   # fp32 bits; PE fast mode (1 cyc/col at N>=256)
    mdt = dt.bfloat16 if mult_bf16 else f32
    pdt = dt.bfloat16 if mult_bf16 else f32r

    assert t_steps % 4 == 0
    nc = bacc.Bacc("TRN2", target_bir_lowering=False, debug=False)

    # ---- DRAM I/O ----
    u_d = nc.dram_tensor("u", [t_steps, C, BS], mdt, kind="ExternalInput")
    f0_d = nc.dram_tensor("f0t", [F, BS], f32r, kind="ExternalInput")
    w2t_d = nc.dram_tensor("w2t", [W, W], f32r, kind="ExternalInput")
    w3t_d = nc.dram_tensor("w3t", [W, HC], f32r, kind="ExternalInput")
    b1_d = nc.dram_tensor("b1c", [W, 1], f32, kind="ExternalInput")
    b2_d = nc.dram_tensor("b2c", [W, 1], f32, kind="ExternalInput")
    b3p_d = nc.dram_tensor("b3p", [2 * NBANK, CH], f32r, kind="ExternalInput")
    o01_d = nc.dram_tensor("o01", [2, 2 * NB], f32r, kind="ExternalInput")
    g1_d = nc.dram_tensor("g1", [CH, NCHUNK * W], f32r, kind="ExternalInput")
    r1p_d = nc.dram_tensor("r1p", [CH, NCHUNK * R1], f32r, kind="ExternalInput")
    wq1_d = nc.dram_tensor("wq1", [F, W], f32r, kind="ExternalInput")
    bq1_d = nc.dram_tensor("bq1", [1, W], f32r, kind="ExternalInput")
    wqr_d = nc.dram_tensor("wqr", [F, R1], f32r, kind="ExternalInput")
    bqr_d = nc.dram_tensor("bqr", [1, R1], f32r, kind="ExternalInput")
    br1_d = nc.dram_tensor("br1c", [R1, 1], f32, kind="ExternalInput")
    wr2t_d = nc.dram_tensor("wr2t", [R1, 1], f32r, kind="ExternalInput")
    br2_d = nc.dram_tensor("br2c", [4, 1], f32, kind="ExternalInput")
    pred_d = nc.dram_tensor("pred", [t_steps, BS], f32, kind="ExternalOutput")

    with tile.TileContext(nc) as tc, ExitStack() as ctx:
        const = ctx.enter_context(tc.tile_pool(name="const", bufs=1))

        def load_const(dram, shape, dtype, tag):
            t_ = const.tile(shape, dtype, tag=tag, name=tag)
            nc.sync.dma_start(t_[:], dram.ap())
            return t_

        w2t_s = load_const(w2t_d, [W, W], f32r, "w2t")
        w3t_s = load_const(w3t_d, [W, HC], f32r, "w3t")
        b1_s = load_const(b1_d, [W, 1], f32, "b1")
        b2_s = load_const(b2_d, [W, 1], f32, "b2")
        # bias pair rows per bank at 32-aligned partitions: bank b at 32*b
        b3p_s = const.tile([34, CH], f32r, tag="b3p", name="b3p")
        nc.sync.dma_start(b3p_s[0:2, :], b3p_d.ap()[0:2, :])
        nc.sync.dma_start(b3p_s[32:34, :], b3p_d.ap()[2:4, :])
        o01_s = const.tile([34, 2 * NB], f32r, tag="o01", name="o01")
        nc.sync.dma_start(o01_s[0:2, :], o01_d.ap())
        nc.sync.dma_start(o01_s[32:34, :], o01_d.ap())
        g1_s = load_const(g1_d, [CH, NCHUNK * W], f32r, "g1")
        r1p_s = load_const(r1p_d, [CH, NCHUNK * R1], f32r, "r1p")
        wq1_s = load_const(wq1_d, [F, W], f32r, "wq1")
        bq1_s = load_const(bq1_d, [1, W], f32r, "bq1")
        wqr_s = load_const(wqr_d, [F, R1], f32r, "wqr")
        bqr_s = load_const(bqr_d, [1, R1], f32r, "bqr")
        br1_s = load_const(br1_d, [R1, 1], f32, "br1")
        wr2t_s = load_const(wr2t_d, [R1, 1], f32r, "wr2t")
        br2_s = load_const(br2_d, [4, 1], f32, "br2")
        f0_s = load_const(f0_d, [F, BS], f32r, "f0")
        ones_f = const.tile([1, NB], f32, tag="ones", name="ones")
        nc.vector.memset(ones_f[:], 1.0)
        ones_s = ones_f[:].bitcast(f32r)

        u_pool = ctx.enter_context(tc.tile_pool(name="upool", bufs=3))
        x_pool = ctx.enter_context(tc.tile_pool(name="xpool", bufs=2))
        m_pool = ctx.enter_context(tc.tile_pool(name="mpool", bufs=2))
        r_pool = ctx.enter_context(tc.tile_pool(name="rpool", bufs=2))
        pred_pool = ctx.enter_context(tc.tile_pool(name="predpool", bufs=1))

        # PSUM: per stream: acc (y1+r1, persistent) 1 bank, y3 2 banks,
        # transient (y2 | r2 rows) 1 bank  -> 4 banks * 2 streams = 8.
        ps_acc, ps_y3, ps_tr = [], [], []
        for s in range(NS):
            ps_acc.append(
                ctx.enter_context(
                    tc.tile_pool(name=f"psacc{s}", bufs=1, space="PSUM")
                )
            )
            ps_y3.append(
                ctx.enter_context(tc.tile_pool(name=f"psy3{s}", bufs=1, space="PSUM"))
            )
            ps_tr.append(
                ctx.enter_context(tc.tile_pool(name=f"pstr{s}", bufs=1, space="PSUM"))
            )
        r2acc = [
            ps_tr[s].tile([4 * R1, NB], f32, tag=f"r2acc{s}", name=f"r2acc{s}")
            for s in range(NS)
        ]
        acc = [
            ps_acc[s].tile([W, 2 * NB], f32, tag=f"acc{s}", name=f"acc{s}")
            for s in range(NS)
        ]
        # acc[s][0:128, 0:NB] = y1 = W1 @ h;  acc[s][0:32, NB:2NB] = r1 = Wr1 @ h

        # ---- seed accumulators: y1 = W1@Wp@f0 + W1@bp, r1 = Wr1@Wp@f0 + Wr1@bp
        for s in range(NS):
            fsl = f0_s[:, s * NB : (s + 1) * NB]
            y1r = acc[s][:, 0:NB]
            r1r = acc[s][0:R1, NB : NB + NB]
            nc.tensor.matmul(y1r, wq1_s[:], fsl, start=True, stop=False)
            nc.tensor.matmul(y1r, bq1_s[:], ones_s, start=False, stop=True)
            nc.tensor.matmul(
                r1r, wqr_s[:], fsl, start=False, stop=False, skip_group_check=True
            )
            nc.tensor.matmul(
                r1r, bqr_s[:], ones_s, start=False, stop=True,
                skip_group_check=True,
            )

        # pred staging: partition = 32*(t%4), columns = (t//4)*BS + b
        n_g = t_steps // 4
        pred_sb = pred_pool.tile([97, n_g * BS], f32, tag="pred", name="pred_sb")

        skew = {}

        def emit_step(s, t):
            u_t = u_pool.tile([CH, NB], mdt, tag=f"u{s}", name=f"u{s}_{t}")
            nc.sync.dma_start(
                u_t[:],
                bass.AP(u_d, (t * C) * BS + s * NB, [[0, CH // C], [BS, C], [1, NB]]),
            )

            # x1 = gelu(y1acc + b1)
            x1 = x_pool.tile([W, NB], f32r, tag=f"x1{s}", name=f"x1_{s}_{t}")
            g1i = nc.scalar.activation(
                x1[:], acc[s][:, 0:NB], AF.Gelu_apprx_tanh, bias=b1_s[:]
            )
            other = 1 - s
            if other in skew:
                tile.add_dep_helper(
                    g1i.ins, skew[other].ins, sync=True,
                    reason="stream anti-phase lock",
                )

            # y2 = W2 @ x1 ; x2 = gelu(y2 + b2)   (y2 borrows the y3-bank0 slot)
            y2 = ps_y3[s].tile([W, NB], f32, tag=f"y3{s}b0", name=f"y2_{s}_{t}")
            nc.tensor.matmul(y2[:], w2t_s[:], x1[:], start=True, stop=True)
            x2 = x_pool.tile([W, NB], f32r, tag=f"x2{s}", name=f"x2_{s}_{t}")
            g2i = nc.scalar.activation(
                x2[:], y2[:], AF.Gelu_apprx_tanh, bias=b2_s[:]
            )
            skew[s] = g2i

            # per-bank wavefront: Y3 bank -> tanh -> P = M*U -> y1/r1 accum
            for bank in range(NBANK):
                j0 = 2 * bank
                y3 = ps_y3[s].tile(
                    [CH, 2 * NB], f32, tag=f"y3{s}b{bank}", name=f"y3_{s}_{bank}_{t}"
                )
                nc.tensor.matmul(
                    y3[:],
                    b3p_s[32 * bank : 32 * bank + 2, :],
                    o01_s[32 * bank : 32 * bank + 2, :],
                    start=True,
                    stop=False,
                )
                for idx, j in enumerate((j0, j0 + 1)):
                    nc.tensor.matmul(
                        y3[:, idx * NB : (idx + 1) * NB],
                        w3t_s[:, j * CH : (j + 1) * CH],
                        x2[:],
                        start=False,
                        stop=(idx == 1),
                    )
                m_t = m_pool.tile(
                    [CH, 2 * NB], mdt, tag=f"m{s}b{bank}", name=f"m_{s}_{bank}_{t}"
                )
                nc.scalar.activation(m_t[:], y3[:], AF.Tanh)
                p_t = m_pool.tile(
                    [CH, 2 * NB], pdt, tag=f"p{s}b{bank}", name=f"p_{s}_{bank}_{t}"
                )
                m3 = m_t[:].rearrange("p (j n) -> p j n", j=2)
                p3 = p_t[:].rearrange("p (j n) -> p j n", j=2)
                u3 = bass.AP(
                    u_t.tensor, u_t.offset, [list(u_t.ap[0]), [0, 2], [1, NB]]
                )
                nc.vector.tensor_tensor(p3, m3, u3, op=ALU.mult)
                for idx, j in enumerate((j0, j0 + 1)):
                    psl = p_t[:, idx * NB : (idx + 1) * NB]
                    last_mm = bank == NBANK - 1 and idx == 1
                    nc.tensor.matmul(
                        acc[s][:, 0:NB],
                        g1_s[:, j * W : (j + 1) * W],
                        psl,
                        start=False,
                        stop=last_mm,
                        skip_group_check=True,
                    )
                    nc.tensor.matmul(
                        acc[s][0:R1, NB : NB + NB],
                        r1p_s[:, j * R1 : (j + 1) * R1],
                        psl,
                        start=False,
                        stop=last_mm,
                        skip_group_check=True,
                    )

            # readout: rl = relu(r1acc + br1); r2 = Wr2 @ rl (+ br2 at evac)
            rl = r_pool.tile([R1, NB], f32r, tag=f"rl{s}", name=f"rl_{s}_{t}")
            nc.vector.tensor_scalar(
                rl[:], acc[s][0:R1, NB : NB + NB], br1_s[:], 0.0,
                op0=ALU.add, op1=ALU.max,
            )
            q = t % 4
            g = t // 4
            nc.tensor.matmul(
                r2acc[s][32 * q : 32 * q + 1, :],
                wr2t_s[:],
                rl[:],
                start=True,
                stop=True,
                tile_position=(0, 32 * q),
            )
            dst = pred_sb[32 * q : 32 * q + 1, g * BS + s * NB : g * BS + (s + 1) * NB]
            nc.vector.tensor_scalar(
                dst, r2acc[s][32 * q : 32 * q + 1, :], br2_s[0:1, :], None, op0=ALU.add
            )

        for t in range(t_steps):
            for s in range(NS):
                emit_step(s, t)

        # final: pred_sb row 32q holds steps t = 4g+q -> pred_d (T, BS)
        for q in range(4):
            nc.sync.dma_start(
                bass.AP(pred_d, q * BS, [[4 * BS, n_g], [1, BS]]),
                pred_sb[32 * q : 32 * q + 1, :].rearrange("p (g b) -> p g b", b=BS),
            )

    nc.compile()
    _BUILD_CACHE[key] = (nc, None)
    return nc, None


def _host_prep(time, features, mask, length, Wp, bp, W1, b1, W2, b2, W3, b3,
               Wr1, br1, Wr2, br2, t_steps=T, mult_bf16=False):
    """Shard + marshal inputs into per-core in_maps."""
    time = np.asarray(time, np.float32)
    features = np.asarray(features, np.float32)
    W1, W2, W3 = (np.asarray(x, np.float32) for x in (W1, W2, W3))
    Wp, Wr1, Wr2 = (np.asarray(x, np.float32) for x in (Wp, Wr1, Wr2))
    b1, b2, b3 = (np.asarray(x, np.float32) for x in (b1, b2, b3))
    bp, br1, br2 = (np.asarray(x, np.float32) for x in (bp, br1, br2))
    mdt = np.float32
    if mult_bf16:
        import ml_dtypes

        mdt = ml_dtypes.bfloat16

    cp = np.concatenate([time[..., None], features], axis=-1)  # (B, Tfull, C)
    cp_next = np.concatenate([cp[:, 1:], cp[:, -1:]], axis=1)
    active = np.arange(cp.shape[1])[None, :] < np.asarray(length)[:, None]
    u_full = ((cp_next - cp) * active[..., None])[:, :t_steps].astype(np.float32)

    # c-contraction patterns A_j (112, 64): A_j[p, h] = 1 iff (112j+p)//7 == h
    a_list = []
    for j in range(NCHUNK):
        a = np.zeros((CH, H), np.float32)
        for p in range(CH):
            a[p, (CH * j + p) // C] = 1.0
        a_list.append(a)

    g1 = np.concatenate([a @ W1.T for a in a_list], axis=1)     # (112, 4*128)
    r1p = np.concatenate([a @ Wr1.T for a in a_list], axis=1)   # (112, 4*32)

    b3p = np.zeros((2 * NBANK, CH), np.float32)                 # rank-2 bias rows
    for bank in range(NBANK):
        b3p[2 * bank] = b3[(2 * bank) * CH : (2 * bank + 1) * CH]
        b3p[2 * bank + 1] = b3[(2 * bank + 1) * CH : (2 * bank + 2) * CH]
    o01 = np.zeros((2, 2 * NB), np.float32)
    o01[0, :NB] = 1.0
    o01[1, NB:] = 1.0

    shared = {
        "w2t": np.ascontiguousarray(W2.T),
        "w3t": np.ascontiguousarray(W3.T),
        "b1c": np.ascontiguousarray(b1.reshape(W, 1)),
        "b2c": np.ascontiguousarray(b2.reshape(W, 1)),
        "b3p": b3p,
        "o01": o01,
        "g1": np.ascontiguousarray(g1),
        "r1p": np.ascontiguousarray(r1p),
        "wq1": np.ascontiguousarray((W1 @ Wp).T),               # (6, 128)
        "bq1": np.ascontiguousarray((W1 @ bp).reshape(1, W)),
        "wqr": np.ascontiguousarray((Wr1 @ Wp).T),              # (6, 32)
        "bqr": np.ascontiguousarray((Wr1 @ bp).reshape(1, R1)),
        "br1c": np.ascontiguousarray(br1.reshape(R1, 1)),
        "wr2t": np.ascontiguousarray(Wr2.T),
        "br2c": np.full((4, 1), np.float32(br2.reshape(-1)[0]), np.float32),
    }

    in_maps = []
    for i in range(NCORES):
        bsl = slice(i * BS, (i + 1) * BS)
        m = dict(shared)
        m["u"] = np.ascontiguousarray(u_full[bsl].transpose(1, 2, 0)).astype(mdt)
        m["f0t"] = np.ascontiguousarray(features[bsl, 0, :].T)
        in_maps.append(m)
    return in_maps


def kernel(**inputs):
    from concourse.bass_utils import run_bass_kernel_spmd

    nc, _ = _build(t_steps=T, mult_bf16=False)
    in_maps = _host_prep(**inputs, t_steps=T, mult_bf16=False)
    res = run_bass_kernel_spmd(nc, in_maps, list(range(NCORES)))
    preds = [res.results[i]["pred"] for i in range(NCORES)]  # (T, BS) each
    out = np.concatenate([p.T for p in preds], axis=0)  # (B, T)
    return np.ascontiguousarray(out.astype(np.float32))
